# revision 27
# baseline (speedup 1.0000x reference)
"""Neural CDE forward pass on 8 Trainium2 NeuronCores (v2).

Model (reference): z0 = coeffs[:,0]@W_init+b_init; RK4 scan over T-1=99 grid
intervals of dz = f(z) dX with f = MLP(64->128->128->128->512) -> tanh ->
reshape [H,C], contracted with dX/dt; then logits/loss/accuracy readout.

Sharding: pure data parallel over batch (2048 -> 8x256). Each core runs the
full scan on its shard; tiny readout done on host from the final z.

v2 design (vs the v1 serial chain at ~27us/step; cost-model ~16.9us/step):
  - Batch shard split into TWO independent chains of 128 columns; per-op
    engine assignment is asymmetric (chain A relus on ScalarE, chain B on
    VectorE) so the chains' elementwise work runs concurrently.
  - fp16 operands for all matmuls (same 10-bit mantissa as the tf32-class
    float32r v1 used; 1 cyc/row at any moving size) and fp16 SBUF
    elementwise (DVE 2x packed mode). Carried state z stays fp32; RK4
    accumulation in fp32.
  - RK4 fused into the tensor engine: the next stage's first-layer psum is
    seeded with W_in^T z and accumulated with a_s (S W_in)^T prod, which
    removes fold->combine->matmul from the critical path (also across the
    step boundary via an fp16 twin of the stage-2 accumulator). The zacc
    path uses w_s-prescaled fold stationaries and one combine per stage.
  - dX/dt is shipped as a tiny [nsteps, 8, 256] fp16 tensor and broadcast
    to the [128, 4, 256] chunk-replicated layout ON DEVICE (4 matmuls + 2
    PSUM->SBUF copies per step), prefetched one step ahead.
  - PSUM discipline (start=True clears a whole bank's has_written bits;
    one live accumulation group per bank): per-chain h1 banks with two
    parity slices also host the layer-2/3 psums; per-chain chunk-pair
    wout banks whose bias seeds (K=2 selector matmuls) run early, off the
    critical path; tanh/prod pipeline per chunk pair.
"""

import numpy as np

from contextlib import ExitStack

from concourse import bacc, mybir
import concourse.tile as tile
from concourse.bass_utils import run_bass_kernel_spmd

N_CORES = 8
B, T, C, H, HH, O = 2048, 100, 8, 64, 128, 10
BS = B // N_CORES          # 256 batch rows per core
CB = BS // 2               # 128 per chain
F32 = mybir.dt.float32
F16 = mybir.dt.float16

ADD = mybir.AluOpType.add
MAX = mybir.AluOpType.max
MULT = mybir.AluOpType.mult
TANH = mybir.ActivationFunctionType.Tanh
RELU = mybir.ActivationFunctionType.Relu
COPY = mybir.ActivationFunctionType.Copy


def _build(nsteps, dts):
    """Build + compile the per-core Bass program. dts: python floats [nsteps]."""
    nc = bacc.Bacc("TRN2", target_bir_lowering=False, debug=False,
                   num_devices=N_CORES)

    def din(name, shape, dt=F16):
        return nc.dram_tensor(name, shape, dt, kind="ExternalInput").ap()

    z0_d = din("z0", [H, BS], F32)
    dxt_d = din("dxt", [nsteps, C, BS])          # f16
    w_in_d = din("w_in", [H, HH])
    w_h0_d = din("w_h0", [HH, HH])
    w_h1_d = din("w_h1", [HH, HH])
    w_out_d = din("w_out", [HH, 4 * HH])         # chunk-permuted
    w2h_d = din("w2h", [HH, HH])                 # (dt/2) * S @ W_in
    w2f_d = din("w2f", [HH, HH])                 # dt * S @ W_in
    w2q_d = din("w2q", [HH, HH])                 # (dt/6) * S @ W_in
    sw16_d = din("sw16", [HH, H])                # (dt/6) * S
    sw13_d = din("sw13", [HH, H])                # (dt/3) * S
    bias01_d = din("bias01", [2, HH])            # output bias, chunks 0,1
    bias23_d = din("bias23", [2, HH])            # output bias, chunks 2,3
    sel2_d = din("sel2", [2, 2 * HH])            # 0/1 chunk-pair selector
    sbc_d = din("sbc", [C, 4 * HH])              # 0/1 dxdt broadcast, 4 chunks
    b_in_d = din("b_in", [HH, 1], F32)
    b_h0_d = din("b_h0", [HH, 1], F32)
    b_h1_d = din("b_h1", [HH, 1], F32)
    zT_d = nc.dram_tensor("zT", [H, BS], F32, kind="ExternalOutput").ap()

    with tile.TileContext(nc) as tc, ExitStack() as ctx:
        const = ctx.enter_context(tc.tile_pool(name="const", bufs=1))

        def load(ap_dram, shape, dt=F16):
            t = const.tile(shape, dt, tag=ap_dram.name)
            nc.sync.dma_start(t[:], ap_dram)
            return t

        w_in = load(w_in_d, [H, HH])
        w_h0 = load(w_h0_d, [HH, HH])
        w_h1 = load(w_h1_d, [HH, HH])
        w_out = load(w_out_d, [HH, 4 * HH])
        w2_half = load(w2h_d, [HH, HH])
        w2_full = load(w2f_d, [HH, HH])
        w2_w4 = load(w2q_d, [HH, HH])
        sw_16 = load(sw16_d, [HH, H])
        sw_13 = load(sw13_d, [HH, H])
        bias01 = load(bias01_d, [2, HH])
        bias23 = load(bias23_d, [2, HH])
        sel2 = load(sel2_d, [2, 2 * HH])
        sbc = load(sbc_d, [C, 4 * HH])
        b_in = load(b_in_d, [HH, 1], F32)
        b_h0 = load(b_h0_d, [HH, 1], F32)
        b_h1 = load(b_h1_d, [HH, 1], F32)

        # SBUF pools
        sb_dx = ctx.enter_context(tc.tile_pool(name="dx", bufs=3))
        sb_d16 = ctx.enter_context(tc.tile_pool(name="d16", bufs=2))
        sb_h = ctx.enter_context(tc.tile_pool(name="h", bufs=2))
        sb_f = ctx.enter_context(tc.tile_pool(name="f", bufs=2))
        sb_p = ctx.enter_context(tc.tile_pool(name="prod", bufs=2))
        sb_z = ctx.enter_context(tc.tile_pool(name="z", bufs=2))
        # PSUM (8 banks of 2KB/partition). Rules respected here: one live
        # accumulation group per bank (start=True clears the whole bank's
        # has_written bits), groups on a bank strictly ordered by data deps.
        #   ph{0,1}: 1 bank each, two [HH,CB] parity slices (cur/next h1)
        #   pfh{0,1}: 1 bank each (layer2 psum, layer3 psum, then wout)
        #   pk{0,1}: 1 bank each (w_s-scaled fold)
        #   dps: 1 bank (dxdt broadcast, two half-passes)
        ps_d = ctx.enter_context(tc.tile_pool(name="psd", bufs=1, space="PSUM"))
        ps_h = ctx.enter_context(tc.tile_pool(name="psh", bufs=1, space="PSUM"))
        ps_f = ctx.enter_context(tc.tile_pool(name="psf", bufs=1, space="PSUM"))
        ps_k = ctx.enter_context(tc.tile_pool(name="psk", bufs=1, space="PSUM"))

        # carried state, both chains side by side: zf (fp32), z16 (fp16)
        zf = sb_z.tile([H, BS], F32, tag="zf", name="zf")
        nc.sync.dma_start(zf[:], z0_d)
        z16 = sb_z.tile([H, BS], F16, tag="z16", name="z16")
        nc.vector.tensor_copy(z16[:], zf[:])

        def emit_dx(ti):
            # dxdt broadcast in two half-passes (one PSUM bank):
            # [8, 256] -> [128, 2, 256] twice, fp16 copies per chain
            dx_t = sb_dx.tile([C, BS], F16, tag="dx", name="dx")
            nc.sync.dma_start(dx_t[:], dxt_d[ti])
            out = [sb_d16.tile([HH, 4, CB], F16, tag=f"d16_{c}", name=f"d16_{c}")
                   for c in (0, 1)]
            for hf in (0, 1):
                d_ps = ps_d.tile([HH, 2, BS], F32, tag="dps", name="dps")
                for j in (0, 1):
                    nc.tensor.matmul(d_ps[:, j, :],
                                     sbc[:, HH * (2 * hf + j):HH * (2 * hf + j + 1)],
                                     dx_t[:], start=(j == 0), stop=(j == 1),
                                     skip_group_check=True)
                nc.scalar.activation(
                    out[0][:, 2 * hf:2 * hf + 2, :],
                    d_ps[:, :, 0:CB], COPY)
                nc.vector.tensor_copy(
                    out[1][:, 2 * hf:2 * hf + 2, :],
                    d_ps[:, :, CB:BS])
            return out

        d16_next = emit_dx(0)

        # per-chain h1-psum bank with two parity slices; stage s reads
        # slice s%2, the fused group for stage s+1 writes slice (s+1)%2
        phb = [None, None]
        for c in (0, 1):
            phb[c] = ps_h.tile([HH, 2, CB], F32, tag=f"ph{c}", name=f"ph{c}")
            nc.tensor.matmul(phb[c][:, 0, :], w_in[:],
                             z16[:, c * CB:(c + 1) * CB],
                             start=True, stop=True, skip_group_check=True)

        par = 0  # parity of the slice holding the CURRENT stage's h1
        for ti in range(nsteps):
            d16 = d16_next
            zacc = None
            for s in range(4):
                # W2 variant used by the h1-accumulate emitted THIS stage
                # (stage s's prod feeds stage s+1's evaluation point):
                #   s=0,1 -> a=dt/2 ; s=2 -> a=dt ; s=3 -> w4=dt/6 into the
                #   NEXT STEP's stage-0 h1 (boundary fusion via zacc2 twin).
                w2_s = (w2_half, w2_half, w2_full, w2_w4)[s]
                sw_s = (sw_16, sw_13, sw_13, sw_16)[s]

                h = [None, None]
                for c in (0, 1):
                    h[c] = sb_h.tile([HH, CB], F16, tag=f"h{c}", name=f"h{c}")
                    if c == 0:
                        nc.scalar.activation(h[c][:], phb[c][:, par, :], RELU,
                                             bias=b_in[:])
                    else:
                        nc.vector.tensor_scalar(h[c][:], phb[c][:, par, :],
                                                b_in[:], 0.0,
                                                op0=ADD, op1=MAX)
                # wout: two single-purpose banks per chain (chunk pairs);
                # each bank's bias seed only waits for the previous stage's
                # tanh read, so emit both up front
                pfh = [[None, None], [None, None]]
                for c in (0, 1):
                    for hp in (0, 1):
                        pfh[c][hp] = ps_f.tile([HH, 2, CB], F32,
                                               tag=f"pf{c}_{hp}", name=f"pf{c}_{hp}")
                        nc.tensor.matmul(pfh[c][hp][:],
                                         bias01[:] if hp == 0 else bias23[:],
                                         sel2[:], start=True, stop=False,
                                         skip_group_check=True)
                # layers 2,3 reuse the ph bank's current-parity slice
                # (free after relu1's read; sequential single-MM groups)
                for c in (0, 1):
                    nc.tensor.matmul(phb[c][:, par, :], w_h0[:], h[c][:],
                                     start=True, stop=True,
                                     skip_group_check=True)
                for c in (0, 1):
                    h[c] = sb_h.tile([HH, CB], F16, tag=f"h{c}", name=f"h{c}")
                    if c == 0:
                        nc.scalar.activation(h[c][:], phb[c][:, par, :], RELU,
                                             bias=b_h0[:])
                    else:
                        nc.vector.tensor_scalar(h[c][:], phb[c][:, par, :],
                                                b_h0[:], 0.0,
                                                op0=ADD, op1=MAX)
                for c in (0, 1):
                    nc.tensor.matmul(phb[c][:, par, :], w_h1[:], h[c][:],
                                     start=True, stop=True,
                                     skip_group_check=True)
                for c in (0, 1):
                    h[c] = sb_h.tile([HH, CB], F16, tag=f"h{c}", name=f"h{c}")
                    if c == 0:
                        nc.scalar.activation(h[c][:], phb[c][:, par, :], RELU,
                                             bias=b_h1[:])
                    else:
                        nc.vector.tensor_scalar(h[c][:], phb[c][:, par, :],
                                                b_h1[:], 0.0,
                                                op0=ADD, op1=MAX)

                # --- output layer chunks accumulate onto the early seeds;
                # tanh/prod pipeline per chunk-pair half
                f_sb = [None, None]
                prod = [None, None]
                for c in (0, 1):
                    f_sb[c] = sb_f.tile([HH, 4, CB], F16, tag=f"f{c}", name=f"f{c}")
                    prod[c] = sb_p.tile([HH, 4, CB], F16, tag=f"prod{c}", name=f"prod{c}")
                for hp in (0, 1):
                    for c in (0, 1):
                        for j in (0, 1):
                            nc.tensor.matmul(pfh[c][hp][:, j, :],
                                             w_out[:, HH * (2 * hp + j):HH * (2 * hp + j + 1)],
                                             h[c][:], start=False, stop=(j == 1),
                                             skip_group_check=True)
                    for c in (0, 1):
                        nc.scalar.activation(f_sb[c][:, 2 * hp:2 * hp + 2, :],
                                             pfh[c][hp][:], TANH)
                    for c in (0, 1):
                        nc.vector.tensor_tensor(
                            prod[c][:, 2 * hp:2 * hp + 2, :],
                            f_sb[c][:, 2 * hp:2 * hp + 2, :],
                            d16[c][:, 2 * hp:2 * hp + 2, :], op=MULT)

                # --- next evaluation point's h1-psum, fused over prod:
                #   ph_next = W_in^T seed16 + a_s (S W_in)^T prod
                # one group per bank (seed start=True clears only the
                # has_written bits; the cur-parity VALUES stay readable)
                for c in (0, 1):
                    if s < 3:
                        seed16 = z16[:, c * CB:(c + 1) * CB]
                    else:
                        seed16 = zacc16[:, c * CB:(c + 1) * CB]
                    nxt = phb[c][:, 1 - par, :]
                    nc.tensor.matmul(nxt, w_in[:], seed16,
                                     start=True, stop=False,
                                     skip_group_check=True)
                    for j in range(4):
                        nc.tensor.matmul(nxt, w2_s[:], prod[c][:, j, :],
                                         start=False, stop=(j == 3),
                                         skip_group_check=True)

                if s == 1 and ti + 1 < nsteps:
                    d16_next = emit_dx(ti + 1)

                # --- zacc fold (w_s-scaled) + RK4 accum. Single 8-MM
                # group on one bank: only the very first MM carries
                # start=True (bank-wide bit clear); chain B's first write
                # lands on cleared bits and overwrite-sets per element.
                # prod[0] always completes before prod[1] (same DVE, in
                # order), so the start MM executes first.
                pk = ps_k.tile([H, 2, CB], F32, tag="pk", name="pk")
                for c in (0, 1):
                    for j in range(4):
                        nc.tensor.matmul(pk[:, c, :], sw_s[:],
                                         prod[c][:, j, :],
                                         start=(c == 0 and j == 0),
                                         stop=(c == 1 and j == 3),
                                         skip_group_check=True)
                if s == 2:
                    # fp16 twin of zacc after stage 2: seeds the boundary
                    # h1 fusion (z_new = zacc2 + w4 k4) -- on the path
                    zacc16 = sb_z.tile([H, BS], F16, tag="za16", name="za16")
                    nc.vector.scalar_tensor_tensor(zacc16[:], pk[:], 1.0,
                                                   zacc[:], op0=MULT, op1=ADD)
                with tc.high_priority(offset=-80):
                    if s == 3:
                        # next step's fp16 state (off critical path now)
                        z16 = sb_z.tile([H, BS], F16, tag="z16", name="z16")
                        nc.vector.scalar_tensor_tensor(z16[:], pk[:], 1.0,
                                                       zacc[:],
                                                       op0=MULT, op1=ADD)
                    zacc_n = sb_z.tile([H, BS], F32, tag="zacc", name="zacc")
                    prev = zf if s == 0 else zacc
                    nc.vector.scalar_tensor_tensor(zacc_n[:], pk[:], 1.0,
                                                   prev[:], op0=MULT, op1=ADD)
                    zacc = zacc_n

                par = 1 - par

            zf = zacc

        nc.sync.dma_start(zT_d, zf[:])

    nc.compile()
    return nc


def _prep_inputs(coeffs, times, W_init, b_init, W_in, b_in, W_h, b_h,
                 W_out, b_out, nsteps):
    """Host-side constants + per-core shards."""
    coeffs = np.asarray(coeffs, np.float32)
    times = np.asarray(times, np.float32)
    dts_full = np.diff(times)
    dxdt = (coeffs[:, 1:, :] - coeffs[:, :-1, :]) / dts_full[None, :, None]
    dts = dts_full[:nsteps]
    dxdt = dxdt[:, :nsteps, :]

    z0 = coeffs[:, 0, :] @ np.asarray(W_init, np.float32) + np.asarray(b_init, np.float32)
    z0 = np.ascontiguousarray(z0.T)  # [H, B]

    p = np.arange(128)
    j = np.arange(4)
    c_idx = 2 * j[None, :] + (p[:, None] >= 64)          # [128, 4]
    col = (p[:, None] % 64) * 8 + c_idx                  # [128, 4] output col

    W_out = np.asarray(W_out, np.float32)                # [HH, 512]
    b_out = np.asarray(b_out, np.float32)                # [512]
    w_out_perm = np.ascontiguousarray(
        W_out[:, col.T.reshape(-1)]).astype(np.float16)  # [HH, (j,p') 512]
    bias4 = np.ascontiguousarray(b_out[col.T]).astype(np.float16)  # [4, 128]

    sel2 = np.zeros((2, 256), np.float16)
    for jj in range(2):
        sel2[jj, 128 * jj:128 * (jj + 1)] = 1.0

    sbc = np.zeros((C, 512), np.float16)                 # [8, (j, p') 512]
    for jj in range(4):
        for pp in range(128):
            sbc[2 * jj + (pp >= 64), 128 * jj + pp] = 1.0

    s_fold = (p[:, None] % 64 == np.arange(H)[None, :]).astype(np.float32)
    dt0 = float(dts[0])
    assert np.allclose(dts, dt0, rtol=1e-5), "kernel assumes a uniform grid"
    W_in32 = np.asarray(W_in, np.float32)
    w2 = np.tile(W_in32, (2, 1))                         # [128, 128] S @ W_in

    W_h = np.asarray(W_h, np.float32)
    b_h = np.asarray(b_h, np.float32)
    consts = {
        "w_in": W_in32.astype(np.float16),
        "w_h0": W_h[0].astype(np.float16),
        "w_h1": W_h[1].astype(np.float16),
        "w_out": w_out_perm,
        "w2h": (0.5 * dt0 * w2).astype(np.float16),
        "w2f": (dt0 * w2).astype(np.float16),
        "w2q": (dt0 / 6.0 * w2).astype(np.float16),
        "sw16": np.ascontiguousarray(dt0 / 6.0 * s_fold).astype(np.float16),
        "sw13": np.ascontiguousarray(dt0 / 3.0 * s_fold).astype(np.float16),
        "bias01": np.ascontiguousarray(bias4[0:2]),
        "bias23": np.ascontiguousarray(bias4[2:4]),
        "sel2": sel2, "sbc": sbc,
        "b_in": np.asarray(b_in, np.float32).reshape(HH, 1).copy(),
        "b_h0": b_h[0].reshape(HH, 1).copy(),
        "b_h1": b_h[1].reshape(HH, 1).copy(),
    }

    in_maps = []
    for ci in range(N_CORES):
        bs, be = ci * BS, (ci + 1) * BS
        dxt = np.ascontiguousarray(
            dxdt[bs:be].transpose(1, 2, 0)).astype(np.float16)  # [nsteps, C, BS]
        m = dict(consts)
        m["z0"] = np.ascontiguousarray(z0[:, bs:be])
        m["dxt"] = dxt
        in_maps.append(m)
    return in_maps, dts


_CACHE = {}
_LAST_RESULTS = None


def _get_nc(nsteps, dts_key, dts):
    key = (nsteps, dts_key)
    if key not in _CACHE:
        _CACHE[key] = _build(nsteps, dts)
    return _CACHE[key]


def run_scan(coeffs, times, W_init, b_init, W_in, b_in, W_h, b_h, W_out, b_out,
             nsteps=None):
    """Run the device scan; returns zT [B, H] float32."""
    times = np.asarray(times, np.float32)
    if nsteps is None:
        nsteps = len(times) - 1
    in_maps, dts = _prep_inputs(coeffs, times, W_init, b_init, W_in, b_in,
                                W_h, b_h, W_out, b_out, nsteps)
    nc = _get_nc(nsteps, dts.tobytes(), dts)
    res = run_bass_kernel_spmd(nc, in_maps, core_ids=list(range(N_CORES)))
    global _LAST_RESULTS
    _LAST_RESULTS = res
    zT = np.concatenate([res.results[ci]["zT"] for ci in range(N_CORES)],
                        axis=1)                          # [H, B]
    return np.ascontiguousarray(zT.T)


def kernel(coeffs, y, times, W_init, b_init, W_in, b_in, W_h, b_h,
           W_out, b_out, W_read, b_read):
    zT = run_scan(coeffs, times, W_init, b_init, W_in, b_in, W_h, b_h,
                  W_out, b_out)
    y = np.asarray(y)
    logits = (zT.astype(np.float64) @ np.asarray(W_read, np.float64)
              + np.asarray(b_read, np.float64))          # [B, O]
    m = logits.max(axis=1, keepdims=True)
    logp = logits - (m + np.log(np.exp(logits - m).sum(axis=1, keepdims=True)))
    loss = np.float32(-logp[np.arange(B), y].mean())
    acc = np.float32((logits.argmax(axis=1) == y).sum())
    return loss, acc


# revision 30
# speedup vs baseline: 1.0189x; 1.0189x over previous
"""Neural CDE forward pass on 8 Trainium2 NeuronCores (v2).

Model (reference): z0 = coeffs[:,0]@W_init+b_init; RK4 scan over T-1=99 grid
intervals of dz = f(z) dX with f = MLP(64->128->128->128->512) -> tanh ->
reshape [H,C], contracted with dX/dt; then logits/loss/accuracy readout.

Sharding: pure data parallel over batch (2048 -> 8x256). Each core runs the
full scan on its shard; tiny readout done on host from the final z.

v2 design (vs the v1 serial chain at ~27us/step; cost-model ~16.9us/step):
  - Batch shard split into TWO independent chains of 128 columns; per-op
    engine assignment is asymmetric (chain A relus on ScalarE, chain B on
    VectorE) so the chains' elementwise work runs concurrently.
  - fp16 operands for all matmuls (same 10-bit mantissa as the tf32-class
    float32r v1 used; 1 cyc/row at any moving size) and fp16 SBUF
    elementwise (DVE 2x packed mode). Carried state z stays fp32; RK4
    accumulation in fp32.
  - RK4 fused into the tensor engine: the next stage's first-layer psum is
    seeded with W_in^T z and accumulated with a_s (S W_in)^T prod, which
    removes fold->combine->matmul from the critical path (also across the
    step boundary via an fp16 twin of the stage-2 accumulator). The zacc
    path uses w_s-prescaled fold stationaries and one combine per stage.
  - dX/dt is shipped as a tiny [nsteps, 8, 256] fp16 tensor and broadcast
    to the [128, 4, 256] chunk-replicated layout ON DEVICE (4 matmuls + 2
    PSUM->SBUF copies per step), prefetched one step ahead.
  - PSUM discipline (start=True clears a whole bank's has_written bits;
    one live accumulation group per bank): per-chain h1 banks with two
    parity slices also host the layer-2/3 psums; per-chain chunk-pair
    wout banks whose bias seeds (K=2 selector matmuls) run early, off the
    critical path; tanh/prod pipeline per chunk pair.
"""

import numpy as np

from contextlib import ExitStack

from concourse import bacc, mybir
import concourse.tile as tile
from concourse.bass_utils import run_bass_kernel_spmd

N_CORES = 8
B, T, C, H, HH, O = 2048, 100, 8, 64, 128, 10
BS = B // N_CORES          # 256 batch rows per core
CB = BS // 2               # 128 per chain
F32 = mybir.dt.float32
F16 = mybir.dt.float16

ADD = mybir.AluOpType.add
MAX = mybir.AluOpType.max
MULT = mybir.AluOpType.mult
TANH = mybir.ActivationFunctionType.Tanh
RELU = mybir.ActivationFunctionType.Relu
COPY = mybir.ActivationFunctionType.Copy


def _build(nsteps, dts):
    """Build + compile the per-core Bass program. dts: python floats [nsteps]."""
    nc = bacc.Bacc("TRN2", target_bir_lowering=False, debug=False,
                   num_devices=N_CORES)

    def din(name, shape, dt=F16):
        return nc.dram_tensor(name, shape, dt, kind="ExternalInput").ap()

    z0_d = din("z0", [H, BS], F32)
    dxt_d = din("dxt", [nsteps, C, BS])          # f16
    w_in_d = din("w_in", [H, HH])
    w_h0_d = din("w_h0", [HH, HH])
    w_h1_d = din("w_h1", [HH, HH])
    w_out_d = din("w_out", [HH, 4 * HH])         # chunk-permuted
    w2h_d = din("w2h", [HH, HH])                 # (dt/2) * S @ W_in
    w2f_d = din("w2f", [HH, HH])                 # dt * S @ W_in
    w2q_d = din("w2q", [HH, HH])                 # (dt/6) * S @ W_in
    sw16_d = din("sw16", [HH, H])                # (dt/6) * S
    sw13_d = din("sw13", [HH, H])                # (dt/3) * S
    bias01_d = din("bias01", [2, HH])            # output bias, chunks 0,1
    bias23_d = din("bias23", [2, HH])            # output bias, chunks 2,3
    sel2_d = din("sel2", [2, 2 * HH])            # 0/1 chunk-pair selector
    sbc_d = din("sbc", [C, 4 * HH])              # 0/1 dxdt broadcast, 4 chunks
    b_in_d = din("b_in", [HH, 1], F32)
    b_h0_d = din("b_h0", [HH, 1], F32)
    b_h1_d = din("b_h1", [HH, 1], F32)
    zT_d = nc.dram_tensor("zT", [H, BS], F32, kind="ExternalOutput").ap()

    with tile.TileContext(nc) as tc, ExitStack() as ctx:
        const = ctx.enter_context(tc.tile_pool(name="const", bufs=1))

        def load(ap_dram, shape, dt=F16):
            t = const.tile(shape, dt, tag=ap_dram.name)
            nc.sync.dma_start(t[:], ap_dram)
            return t

        w_in = load(w_in_d, [H, HH])
        w_h0 = load(w_h0_d, [HH, HH])
        w_h1 = load(w_h1_d, [HH, HH])
        w_out = load(w_out_d, [HH, 4 * HH])
        w2_half = load(w2h_d, [HH, HH])
        w2_full = load(w2f_d, [HH, HH])
        w2_w4 = load(w2q_d, [HH, HH])
        sw_16 = load(sw16_d, [HH, H])
        sw_13 = load(sw13_d, [HH, H])
        bias01 = load(bias01_d, [2, HH])
        bias23 = load(bias23_d, [2, HH])
        sel2 = load(sel2_d, [2, 2 * HH])
        sbc = load(sbc_d, [C, 4 * HH])
        b_in = load(b_in_d, [HH, 1], F32)
        b_h0 = load(b_h0_d, [HH, 1], F32)
        b_h1 = load(b_h1_d, [HH, 1], F32)

        # SBUF pools
        sb_dx = ctx.enter_context(tc.tile_pool(name="dx", bufs=3))
        sb_d16 = ctx.enter_context(tc.tile_pool(name="d16", bufs=3))
        sb_h = ctx.enter_context(tc.tile_pool(name="h", bufs=4))
        sb_f = ctx.enter_context(tc.tile_pool(name="f", bufs=3))
        sb_p = ctx.enter_context(tc.tile_pool(name="prod", bufs=3))
        sb_z = ctx.enter_context(tc.tile_pool(name="z", bufs=3))
        # PSUM (8 banks of 2KB/partition). Rules respected here: one live
        # accumulation group per bank (start=True clears the whole bank's
        # has_written bits), groups on a bank strictly ordered by data deps.
        #   ph{0,1}: 1 bank each, two [HH,CB] parity slices (cur/next h1)
        #   pfh{0,1}: 1 bank each (layer2 psum, layer3 psum, then wout)
        #   pk{0,1}: 1 bank each (w_s-scaled fold)
        #   dps: 1 bank (dxdt broadcast, two half-passes)
        ps_d = ctx.enter_context(tc.tile_pool(name="psd", bufs=1, space="PSUM"))
        ps_h = ctx.enter_context(tc.tile_pool(name="psh", bufs=1, space="PSUM"))
        ps_f = ctx.enter_context(tc.tile_pool(name="psf", bufs=1, space="PSUM"))
        ps_k = ctx.enter_context(tc.tile_pool(name="psk", bufs=1, space="PSUM"))

        # carried state, both chains side by side: zf (fp32), z16 (fp16)
        zf = sb_z.tile([H, BS], F32, tag="zf", name="zf")
        nc.sync.dma_start(zf[:], z0_d)
        z16 = sb_z.tile([H, BS], F16, tag="z16", name="z16")
        nc.vector.tensor_copy(z16[:], zf[:])

        def emit_dx(ti):
            # dxdt broadcast in two half-passes (one PSUM bank):
            # [8, 256] -> [128, 2, 256] twice, fp16 copies per chain
            dx_t = sb_dx.tile([C, BS], F16, tag="dx", name="dx")
            nc.sync.dma_start(dx_t[:], dxt_d[ti])
            out = [sb_d16.tile([HH, 4, CB], F16, tag=f"d16_{c}", name=f"d16_{c}")
                   for c in (0, 1)]
            for hf in (0, 1):
                d_ps = ps_d.tile([HH, 2, BS], F32, tag="dps", name="dps")
                for j in (0, 1):
                    nc.tensor.matmul(d_ps[:, j, :],
                                     sbc[:, HH * (2 * hf + j):HH * (2 * hf + j + 1)],
                                     dx_t[:], start=(j == 0), stop=(j == 1),
                                     skip_group_check=True)
                nc.scalar.activation(
                    out[0][:, 2 * hf:2 * hf + 2, :],
                    d_ps[:, :, 0:CB], COPY)
                nc.vector.tensor_copy(
                    out[1][:, 2 * hf:2 * hf + 2, :],
                    d_ps[:, :, CB:BS])
            return out

        d16_next = emit_dx(0)

        # per-chain h1-psum bank with two parity slices; stage s reads
        # slice s%2, the fused group for stage s+1 writes slice (s+1)%2
        phb = [None, None]
        for c in (0, 1):
            phb[c] = ps_h.tile([HH, 2, CB], F32, tag=f"ph{c}", name=f"ph{c}")
            nc.tensor.matmul(phb[c][:, 0, :], w_in[:],
                             z16[:, c * CB:(c + 1) * CB],
                             start=True, stop=True, skip_group_check=True)

        par = 0  # parity of the slice holding the CURRENT stage's h1
        for ti in range(nsteps):
            d16 = d16_next
            zacc = None
            for s in range(4):
                # W2 variant used by the h1-accumulate emitted THIS stage
                # (stage s's prod feeds stage s+1's evaluation point):
                #   s=0,1 -> a=dt/2 ; s=2 -> a=dt ; s=3 -> w4=dt/6 into the
                #   NEXT STEP's stage-0 h1 (boundary fusion via zacc2 twin).
                w2_s = (w2_half, w2_half, w2_full, w2_w4)[s]
                sw_s = (sw_16, sw_13, sw_13, sw_16)[s]

                h = [None, None]
                for c in (0, 1):
                    h[c] = sb_h.tile([HH, CB], F16, tag=f"h{c}", name=f"h{c}")
                    if c == 0:
                        nc.scalar.activation(h[c][:], phb[c][:, par, :], RELU,
                                             bias=b_in[:])
                    else:
                        nc.vector.tensor_scalar(h[c][:], phb[c][:, par, :],
                                                b_in[:], 0.0,
                                                op0=ADD, op1=MAX)
                # wout: two single-purpose banks per chain (chunk pairs);
                # each bank's bias seed only waits for the previous stage's
                # tanh read, so emit both up front
                pfh = [[None, None], [None, None]]
                for c in (0, 1):
                    for hp in (0, 1):
                        pfh[c][hp] = ps_f.tile([HH, 2, CB], F32,
                                               tag=f"pf{c}_{hp}", name=f"pf{c}_{hp}")
                        nc.tensor.matmul(pfh[c][hp][:],
                                         bias01[:] if hp == 0 else bias23[:],
                                         sel2[:], start=True, stop=False,
                                         skip_group_check=True)
                # layers 2,3 reuse the ph bank's current-parity slice
                # (free after relu1's read; sequential single-MM groups)
                for c in (0, 1):
                    nc.tensor.matmul(phb[c][:, par, :], w_h0[:], h[c][:],
                                     start=True, stop=True,
                                     skip_group_check=True)
                for c in (0, 1):
                    h[c] = sb_h.tile([HH, CB], F16, tag=f"h{c}", name=f"h{c}")
                    if c == 0:
                        nc.scalar.activation(h[c][:], phb[c][:, par, :], RELU,
                                             bias=b_h0[:])
                    else:
                        nc.vector.tensor_scalar(h[c][:], phb[c][:, par, :],
                                                b_h0[:], 0.0,
                                                op0=ADD, op1=MAX)
                for c in (0, 1):
                    nc.tensor.matmul(phb[c][:, par, :], w_h1[:], h[c][:],
                                     start=True, stop=True,
                                     skip_group_check=True)
                for c in (0, 1):
                    h[c] = sb_h.tile([HH, CB], F16, tag=f"h{c}", name=f"h{c}")
                    if c == 0:
                        nc.scalar.activation(h[c][:], phb[c][:, par, :], RELU,
                                             bias=b_h1[:])
                    else:
                        nc.vector.tensor_scalar(h[c][:], phb[c][:, par, :],
                                                b_h1[:], 0.0,
                                                op0=ADD, op1=MAX)

                # --- output layer chunks accumulate onto the early seeds;
                # tanh/prod pipeline per chunk-pair half
                f_sb = [None, None]
                prod = [None, None]
                for c in (0, 1):
                    f_sb[c] = sb_f.tile([HH, 4, CB], F16, tag=f"f{c}", name=f"f{c}")
                    prod[c] = sb_p.tile([HH, 4, CB], F16, tag=f"prod{c}", name=f"prod{c}")
                for hp in (0, 1):
                    for c in (0, 1):
                        for j in (0, 1):
                            nc.tensor.matmul(pfh[c][hp][:, j, :],
                                             w_out[:, HH * (2 * hp + j):HH * (2 * hp + j + 1)],
                                             h[c][:], start=False, stop=(j == 1),
                                             skip_group_check=True)
                    for c in (0, 1):
                        nc.scalar.activation(f_sb[c][:, 2 * hp:2 * hp + 2, :],
                                             pfh[c][hp][:], TANH)
                    for c in (0, 1):
                        nc.vector.tensor_tensor(
                            prod[c][:, 2 * hp:2 * hp + 2, :],
                            f_sb[c][:, 2 * hp:2 * hp + 2, :],
                            d16[c][:, 2 * hp:2 * hp + 2, :], op=MULT)

                # --- next evaluation point's h1-psum, fused over prod:
                #   ph_next = W_in^T seed16 + a_s (S W_in)^T prod
                # one group per bank (seed start=True clears only the
                # has_written bits; the cur-parity VALUES stay readable)
                for c in (0, 1):
                    if s < 3:
                        seed16 = z16[:, c * CB:(c + 1) * CB]
                    else:
                        seed16 = zacc16[:, c * CB:(c + 1) * CB]
                    nxt = phb[c][:, 1 - par, :]
                    nc.tensor.matmul(nxt, w_in[:], seed16,
                                     start=True, stop=False,
                                     skip_group_check=True)
                    for j in range(4):
                        nc.tensor.matmul(nxt, w2_s[:], prod[c][:, j, :],
                                         start=False, stop=(j == 3),
                                         skip_group_check=True)

                if s == 1 and ti + 1 < nsteps:
                    d16_next = emit_dx(ti + 1)

                # --- zacc fold (w_s-scaled) + RK4 accum. Single 8-MM
                # group on one bank: only the very first MM carries
                # start=True (bank-wide bit clear); chain B's first write
                # lands on cleared bits and overwrite-sets per element.
                # prod[0] always completes before prod[1] (same DVE, in
                # order), so the start MM executes first.
                pk = ps_k.tile([H, 2, CB], F32, tag="pk", name="pk")
                for c in (0, 1):
                    for j in range(4):
                        nc.tensor.matmul(pk[:, c, :], sw_s[:],
                                         prod[c][:, j, :],
                                         start=(c == 0 and j == 0),
                                         stop=(c == 1 and j == 3),
                                         skip_group_check=True)
                if s == 2:
                    # fp16 twin of zacc after stage 2: seeds the boundary
                    # h1 fusion (z_new = zacc2 + w4 k4) -- on the path
                    zacc16 = sb_z.tile([H, BS], F16, tag="za16", name="za16")
                    nc.vector.scalar_tensor_tensor(zacc16[:], pk[:], 1.0,
                                                   zacc[:], op0=MULT, op1=ADD)
                with tc.high_priority(offset=-80):
                    if s == 3:
                        # next step's fp16 state (off critical path now)
                        z16 = sb_z.tile([H, BS], F16, tag="z16", name="z16")
                        nc.vector.scalar_tensor_tensor(z16[:], pk[:], 1.0,
                                                       zacc[:],
                                                       op0=MULT, op1=ADD)
                    zacc_n = sb_z.tile([H, BS], F32, tag="zacc", name="zacc")
                    prev = zf if s == 0 else zacc
                    nc.vector.scalar_tensor_tensor(zacc_n[:], pk[:], 1.0,
                                                   prev[:], op0=MULT, op1=ADD)
                    zacc = zacc_n

                par = 1 - par

            zf = zacc

        nc.sync.dma_start(zT_d, zf[:])

    nc.compile()
    return nc


def _prep_inputs(coeffs, times, W_init, b_init, W_in, b_in, W_h, b_h,
                 W_out, b_out, nsteps):
    """Host-side constants + per-core shards."""
    coeffs = np.asarray(coeffs, np.float32)
    times = np.asarray(times, np.float32)
    dts_full = np.diff(times)
    dxdt = (coeffs[:, 1:, :] - coeffs[:, :-1, :]) / dts_full[None, :, None]
    dts = dts_full[:nsteps]
    dxdt = dxdt[:, :nsteps, :]

    z0 = coeffs[:, 0, :] @ np.asarray(W_init, np.float32) + np.asarray(b_init, np.float32)
    z0 = np.ascontiguousarray(z0.T)  # [H, B]

    p = np.arange(128)
    j = np.arange(4)
    c_idx = 2 * j[None, :] + (p[:, None] >= 64)          # [128, 4]
    col = (p[:, None] % 64) * 8 + c_idx                  # [128, 4] output col

    W_out = np.asarray(W_out, np.float32)                # [HH, 512]
    b_out = np.asarray(b_out, np.float32)                # [512]
    w_out_perm = np.ascontiguousarray(
        W_out[:, col.T.reshape(-1)]).astype(np.float16)  # [HH, (j,p') 512]
    bias4 = np.ascontiguousarray(b_out[col.T]).astype(np.float16)  # [4, 128]

    sel2 = np.zeros((2, 256), np.float16)
    for jj in range(2):
        sel2[jj, 128 * jj:128 * (jj + 1)] = 1.0

    sbc = np.zeros((C, 512), np.float16)                 # [8, (j, p') 512]
    for jj in range(4):
        for pp in range(128):
            sbc[2 * jj + (pp >= 64), 128 * jj + pp] = 1.0

    s_fold = (p[:, None] % 64 == np.arange(H)[None, :]).astype(np.float32)
    dt0 = float(dts[0])
    assert np.allclose(dts, dt0, rtol=1e-5), "kernel assumes a uniform grid"
    W_in32 = np.asarray(W_in, np.float32)
    w2 = np.tile(W_in32, (2, 1))                         # [128, 128] S @ W_in

    W_h = np.asarray(W_h, np.float32)
    b_h = np.asarray(b_h, np.float32)
    consts = {
        "w_in": W_in32.astype(np.float16),
        "w_h0": W_h[0].astype(np.float16),
        "w_h1": W_h[1].astype(np.float16),
        "w_out": w_out_perm,
        "w2h": (0.5 * dt0 * w2).astype(np.float16),
        "w2f": (dt0 * w2).astype(np.float16),
        "w2q": (dt0 / 6.0 * w2).astype(np.float16),
        "sw16": np.ascontiguousarray(dt0 / 6.0 * s_fold).astype(np.float16),
        "sw13": np.ascontiguousarray(dt0 / 3.0 * s_fold).astype(np.float16),
        "bias01": np.ascontiguousarray(bias4[0:2]),
        "bias23": np.ascontiguousarray(bias4[2:4]),
        "sel2": sel2, "sbc": sbc,
        "b_in": np.asarray(b_in, np.float32).reshape(HH, 1).copy(),
        "b_h0": b_h[0].reshape(HH, 1).copy(),
        "b_h1": b_h[1].reshape(HH, 1).copy(),
    }

    in_maps = []
    for ci in range(N_CORES):
        bs, be = ci * BS, (ci + 1) * BS
        dxt = np.ascontiguousarray(
            dxdt[bs:be].transpose(1, 2, 0)).astype(np.float16)  # [nsteps, C, BS]
        m = dict(consts)
        m["z0"] = np.ascontiguousarray(z0[:, bs:be])
        m["dxt"] = dxt
        in_maps.append(m)
    return in_maps, dts


_CACHE = {}
_LAST_RESULTS = None


def _get_nc(nsteps, dts_key, dts):
    key = (nsteps, dts_key)
    if key not in _CACHE:
        _CACHE[key] = _build(nsteps, dts)
    return _CACHE[key]


def run_scan(coeffs, times, W_init, b_init, W_in, b_in, W_h, b_h, W_out, b_out,
             nsteps=None):
    """Run the device scan; returns zT [B, H] float32."""
    times = np.asarray(times, np.float32)
    if nsteps is None:
        nsteps = len(times) - 1
    in_maps, dts = _prep_inputs(coeffs, times, W_init, b_init, W_in, b_in,
                                W_h, b_h, W_out, b_out, nsteps)
    nc = _get_nc(nsteps, dts.tobytes(), dts)
    res = run_bass_kernel_spmd(nc, in_maps, core_ids=list(range(N_CORES)))
    global _LAST_RESULTS
    _LAST_RESULTS = res
    zT = np.concatenate([res.results[ci]["zT"] for ci in range(N_CORES)],
                        axis=1)                          # [H, B]
    return np.ascontiguousarray(zT.T)


def kernel(coeffs, y, times, W_init, b_init, W_in, b_in, W_h, b_h,
           W_out, b_out, W_read, b_read):
    zT = run_scan(coeffs, times, W_init, b_init, W_in, b_in, W_h, b_h,
                  W_out, b_out)
    y = np.asarray(y)
    logits = (zT.astype(np.float64) @ np.asarray(W_read, np.float64)
              + np.asarray(b_read, np.float64))          # [B, O]
    m = logits.max(axis=1, keepdims=True)
    logp = logits - (m + np.log(np.exp(logits - m).sum(axis=1, keepdims=True)))
    loss = np.float32(-logp[np.arange(B), y].mean())
    acc = np.float32((logits.argmax(axis=1) == y).sum())
    return loss, acc
